# revision 32
# baseline (speedup 1.0000x reference)
"""MoE routing kernel for Trainium2 (8 NeuronCores, SPMD expert-parallel).

Contract: kernel(**full_inputs) -> full output [B, S, H] float32.

Strategy
--------
- Host: compute the (tiny) gate + group-topk routing in numpy (bit-identical
  selection to the jax reference), build the per-(token,expert) combine
  weights, and dispatch: gather each expert's tokens into a padded,
  transposed bf16 buffer.  This is the "all-to-all by topk_idx" of the
  sharding hint, done at input-sharding time.
- Device (SPMD over 8 cores): core c holds experts (2c, 2c+1) and a 1/8
  TOKEN-slice of the shared expert.  The routed phase runs first: the
  SwiGLU MLP for its two experts over their gathered tokens (unweighted).
  The shared phase runs last over the core's 512-token slice with the FULL
  shared weights: SI=2816 tiles exactly as 22x128 (no column padding, no
  per-block weight reloads), and the final output drain is one small ys
  tile instead of a bulk write.
- Host: scale per-expert outputs by routing weights, scatter-add over
  token indices, place each core's shared token-slice, transpose back.

All matmuls run in bf16 with fp32 PSUM accumulation.  Weight panels are
pre-tiled on the host into the exact SBUF tile layout so each streams from
HBM exactly once as a contiguous per-partition DMA.  DMA queue discipline:
loads ride scalar/sync, routed-output writes + shared-down loads ride
gpsimd, ys writes ride scalar after every load trigger — a waiting write
never delays an urgent load.
"""

import math

import numpy as np
import ml_dtypes

H = 2048          # hidden size
I = 1408          # intermediate per routed expert
E = 16            # routed experts
G = 4             # groups
TOPK_GROUP = 2
TOP_K = 6
N_SHARED = 2
SCALE_FACTOR = 2.5
SI = I * N_SHARED  # 2816 shared intermediate
N_CORES = 8
EXP_PER_CORE = E // N_CORES  # 2
P = 128
BF16 = ml_dtypes.bfloat16

_COMPILED = {}  # (T, caps) -> nc
_LAST = {}      # debug/profiling handle for test.py


def _gate_host(hs, gate_weight, bias):
    """numpy replica of reference._gate (verified bit-identical selection)."""
    T = hs.shape[0]
    logits = hs @ gate_weight.T                       # [T, E] fp32
    scores = 1.0 / (1.0 + np.exp(-logits))
    sfc = scores + bias[None, :]
    gs = sfc.reshape(T, G, E // G)
    gsort = np.sort(gs, axis=-1)
    group_scores = gsort[..., -1] + gsort[..., -2]
    group_idx = np.argsort(-group_scores, axis=-1, kind="stable")[:, :TOPK_GROUP]
    gmask = np.zeros((T, G), bool)
    gmask[np.arange(T)[:, None], group_idx] = True
    smask = np.repeat(gmask, E // G, axis=1)
    tmp = np.where(smask, sfc, 0.0)
    topk_idx = np.argsort(-tmp, axis=-1, kind="stable")[:, :TOP_K]
    topk_w = np.take_along_axis(scores, topk_idx, axis=1)
    topk_w = topk_w / (topk_w.sum(-1, keepdims=True) + 1e-20) * SCALE_FACTOR
    return topk_idx.astype(np.int32), topk_w.astype(np.float32)


def _build(T, caps):
    """Build + compile the SPMD Bass program.

    T    : total tokens; each core's shared slice is TS = T/8 of them
    caps : per expert slot, (C_cap, w): gathered-token capacity and matmul
           free-dim slice width; C_cap = NP_R * 2 * w
    """
    import concourse.mybir as mybir
    import concourse.tile as tile
    from concourse import bacc

    bf = mybir.dt.bfloat16
    f32 = mybir.dt.float32
    AF = mybir.ActivationFunctionType

    KH = H // P        # 16 contraction chunks over H
    MI = I // P        # 11 I chunks
    MH = H // P        # 16 output H chunks
    MSI = SI // (2 * P)   # 11 shared-intermediate chunks (half columns)
    TS = T // 4           # 1024 token slice for the shared expert (D=2)
    assert TS == 1024
    for (C_cap, w) in caps:
        assert C_cap % (2 * w) == 0 and w <= 512
    C_tot = sum(C_cap for C_cap, _ in caps)
    slot_base = [sum(C for C, _ in caps[:s]) for s in range(len(caps))]
    w_max = max(w for _, w in caps)

    nc = bacc.Bacc("TRN2", target_bir_lowering=False, debug=False,
                   num_devices=N_CORES)
    # x panels are host-packed to [tile, p, kk*cols+c] so each tile loads as
    # one DMA with fat (multi-KB) contiguous per-partition rows
    xs = nc.dram_tensor("xs", [KH // 4, P, 4 * TS], bf, kind="ExternalInput")
    xgs = [
        nc.dram_tensor(f"xg{s}", [(C // (2 * w)) * (KH // 4), P, 4 * 2 * w],
                       bf, kind="ExternalInput")
        for s, (C, w) in enumerate(caps)
    ]
    # weight panels are pre-tiled on the host to the exact SBUF tile layout
    # [tile_idx, partition, ko*128+c] so every load is a contiguous
    # per-partition stream
    wg = nc.dram_tensor("wg", [EXP_PER_CORE * MI, P, KH * P], bf,
                        kind="ExternalInput")
    wu = nc.dram_tensor("wu", [EXP_PER_CORE * MI, P, KH * P], bf,
                        kind="ExternalInput")
    wd = nc.dram_tensor("wd", [EXP_PER_CORE * MH, P, MI * P], bf,
                        kind="ExternalInput")
    sg = nc.dram_tensor("sg", [MSI, P, KH * P], bf, kind="ExternalInput")
    su = nc.dram_tensor("su", [MSI, P, KH * P], bf, kind="ExternalInput")
    sd = nc.dram_tensor("sd", [MH, P, MSI * P], bf, kind="ExternalInput")
    ye = nc.dram_tensor("ye", [H, C_tot], bf, kind="ExternalOutput")
    ys = nc.dram_tensor("ys", [H, TS], bf, kind="ExternalOutput")

    KP = KH // 2       # 8 k-pair x tiles per np block

    with tile.TileContext(nc) as tc:
        with (
            tc.tile_pool(name="xgp", bufs=9) as xgp,   # [128,4,2w] gathered x
            tc.tile_pool(name="xsp", bufs=4) as xsp,   # [128,4,TS] shared x
            tc.tile_pool(name="wp", bufs=8) as wp,     # [128,16,128] weight cols
            tc.tile_pool(name="wdp", bufs=4) as wdp,   # [128,11,128] down cols
            tc.tile_pool(name="sdp", bufs=3) as sdp,   # [128,22,128] shared down
            tc.tile_pool(name="itp", bufs=44) as itp,  # [128,512] bf16 inter
            tc.tile_pool(name="tmp", bufs=2) as tmp,   # silu temp
            tc.tile_pool(name="otp", bufs=3) as otp,   # [128,1024] bf16 out
            tc.tile_pool(name="pg", bufs=3, space="PSUM") as pgp,
            tc.tile_pool(name="pu", bufs=3, space="PSUM") as pup,
            tc.tile_pool(name="py", bufs=2, space="PSUM") as pyp,
        ):
            # Queue discipline: gpsimd carries ONLY data-dependent writes
            # (plus the late sd loads behind them), so no load trigger ever
            # queues behind a write's semaphore wait.  Loads ride
            # scalar+sync; slot 1's x allocations are throttled by the xgp
            # pool (bufs=17) so their DMAs fire right when slot 0's gate/up
            # finishes, landing during slot 0's down phase.
            def load_w0(s, wait=None, split=False):
                # m=0 gate/up weights, emitted BEFORE the x stream so the
                # first chain's stationary tiles land first; the warmup
                # slot splits the gate tile in half so chain k=0 can fire
                # after only 0.26MB
                with tc.tile_wait_until(wait, enable=wait is not None):
                    if split:
                        wga = wp.tile([P, KH // 2, P], bf, name=f"wgt{s}_0a",
                                      tag="wp")
                        nc.sync.dma_start(
                            wga[:], wg[s * MI][:, :KH // 2 * P]
                            .rearrange("p (ko c) -> p ko c", c=P))
                        wgb = wp.tile([P, KH // 2, P], bf, name=f"wgt{s}_0b",
                                      tag="wp")
                        nc.sync.dma_start(
                            wgb[:], wg[s * MI][:, KH // 2 * P:]
                            .rearrange("p (ko c) -> p ko c", c=P))
                        wgt = ("split", wga, wgb)
                    else:
                        wgt = wp.tile([P, KH, P], bf, name=f"wgt{s}_0",
                                      tag="wp")
                        nc.sync.dma_start(
                            wgt[:],
                            wg[s * MI].rearrange("p (ko c) -> p ko c", c=P))
                    wut = wp.tile([P, KH, P], bf, name=f"wut{s}_0", tag="wp")
                    nc.sync.dma_start(
                        wut[:], wu[s * MI].rearrange("p (ko c) -> p ko c", c=P))
                return wgt, wut

            def load_x(s, wait=None, with_w1=False):
                C_cap, w = caps[s]
                NP_R = C_cap // (2 * w)
                xgt = {}
                w1 = None
                with tc.tile_wait_until(wait, enable=wait is not None):
                    for np_ in range(NP_R):
                        for kq in range(KH // 4):
                            if with_w1 and np_ == 0 and kq == 0:
                                # warmup: halve the first quad so chain k=0
                                # fires after 0.41MB instead of 0.82MB
                                ta = xgp.tile([P, 4, 2 * w_max], bf,
                                              name=f"xg{s}_0_0a", tag="x")
                                nc.scalar.dma_start(
                                    ta[:, :2, :2 * w],
                                    xgs[s][0][:, :2 * 2 * w]
                                    .rearrange("p (kk c) -> p kk c", c=2 * w))
                                tb = xgp.tile([P, 4, 2 * w_max], bf,
                                              name=f"xg{s}_0_0b", tag="x")
                                nc.scalar.dma_start(
                                    tb[:, :2, :2 * w],
                                    xgs[s][0][:, 2 * 2 * w:]
                                    .rearrange("p (kk c) -> p kk c", c=2 * w))
                                xgt[(np_, kq)] = ("split", ta, tb)
                                # m=1 weights ride scalar right behind the
                                # first x pair (needed ~11us in)
                                wgt = wp.tile([P, KH, P], bf,
                                              name=f"wgt{s}_1", tag="wp")
                                nc.scalar.dma_start(
                                    wgt[:], wg[s * MI + 1]
                                    .rearrange("p (ko c) -> p ko c", c=P))
                                wut = wp.tile([P, KH, P], bf,
                                              name=f"wut{s}_1", tag="wp")
                                nc.scalar.dma_start(
                                    wut[:], wu[s * MI + 1]
                                    .rearrange("p (ko c) -> p ko c", c=P))
                                w1 = (wgt, wut)
                                continue
                            t = xgp.tile([P, 4, 2 * w_max], bf,
                                         name=f"xg{s}_{np_}_{kq}", tag="x")
                            eng = (nc.scalar, nc.sync)[kq % 2]
                            eng.dma_start(
                                t[:, :, :2 * w],
                                xgs[s][np_ * (KH // 4) + kq]
                                .rearrange("p (kk c) -> p kk c", c=2 * w))
                            xgt[(np_, kq)] = t
                            if with_w1 and np_ == 0 and kq == 0:
                                # m=1 weights ride scalar right behind the
                                # first x quad: needed ~11us in by the
                                # warm-up chain schedule (m0,np0),(m1,np0)
                                wgt = wp.tile([P, KH, P], bf,
                                              name=f"wgt{s}_1", tag="wp")
                                nc.scalar.dma_start(
                                    wgt[:], wg[s * MI + 1]
                                    .rearrange("p (ko c) -> p ko c", c=P))
                                wut = wp.tile([P, KH, P], bf,
                                              name=f"wut{s}_1", tag="wp")
                                nc.scalar.dma_start(
                                    wut[:], wu[s * MI + 1]
                                    .rearrange("p (ko c) -> p ko c", c=P))
                                w1 = (wgt, wut)
                return (xgt, w1) if with_w1 else xgt

            def gate_up(s, xgt, w0, w_pre=None):
                C_cap, w = caps[s]
                NP_R = C_cap // (2 * w)
                inter = {}
                wtiles = {(m, 0): v for m, v in (w_pre or {}).items()}

                def wts(m, sweep=0):
                    if m == 0 and sweep == 0:
                        return w0
                    if (m, sweep) not in wtiles:
                        wgt = wp.tile([P, KH, P], bf,
                                      name=f"wgt{s}_{m}_{sweep}", tag="wp")
                        nc.sync.dma_start(
                            wgt[:],
                            wg[s * MI + m].rearrange("p (ko c) -> p ko c", c=P))
                        wut = wp.tile([P, KH, P], bf,
                                      name=f"wut{s}_{m}_{sweep}", tag="wp")
                        nc.sync.dma_start(
                            wut[:],
                            wu[s * MI + m].rearrange("p (ko c) -> p ko c", c=P))
                        wtiles[(m, sweep)] = (wgt, wut)
                    return wtiles[(m, sweep)]

                # slot 0 warm-up: sweep ALL of np block 0 first (3.3MB of
                # x instead of 6.6MB before the PE is fully fed), then sweep
                # np block 1 with reloaded weights (+11.5MB spread over the
                # whole sweep -- cheap at ~100GB/s)
                if s == 0 and NP_R == 2:
                    sched = [(m, 0) for m in range(MI)] + [
                        (m, 1) for m in range(MI)]
                else:
                    sched = [(m, np_) for m in range(MI)
                             for np_ in range(NP_R)]
                def xop(np_, k, j):
                    t = xgt[(np_, k // 4)]
                    if isinstance(t, tuple):
                        half = t[1] if k % 4 < 2 else t[2]
                        return half[:, k % 2, j * w:(j + 1) * w]
                    return t[:, k % 4, j * w:(j + 1) * w]

                def wop(wgt, k):
                    if isinstance(wgt, tuple):
                        half = wgt[1] if k < KH // 2 else wgt[2]
                        return half[:, k % (KH // 2), :]
                    return wgt[:, k, :]

                for m, np_ in sched:
                    sweep = np_ if (s == 0 and NP_R == 2) else 0
                    wgt, wut = wts(m, sweep)
                    if True:
                        for j in range(2):
                            psg = pgp.tile([P, 512], f32,
                                           name=f"psg{s}_{m}_{np_}{j}",
                                           tag="pg")
                            for k in range(KH):
                                nc.tensor.matmul(
                                    psg[:, :w], wop(wgt, k),
                                    xop(np_, k, j),
                                    start=(k == 0), stop=(k == KH - 1))
                            st = tmp.tile([P, 512], bf,
                                          name=f"st{s}_{m}_{np_}{j}",
                                          tag="tmp")
                            nc.scalar.activation(st[:, :w], psg[:, :w],
                                                 AF.Silu)
                            psu = pup.tile([P, 512], f32,
                                           name=f"psu{s}_{m}_{np_}{j}",
                                           tag="pu")
                            for k in range(KH):
                                nc.tensor.matmul(
                                    psu[:, :w], wut[:, k, :],
                                    xop(np_, k, j),
                                    start=(k == 0), stop=(k == KH - 1))
                            it = itp.tile([P, 512], bf,
                                          name=f"it{s}_{m}_{np_}{j}",
                                          tag="it")
                            nc.vector.tensor_mul(it[:, :w], st[:, :w],
                                                 psu[:, :w])
                            inter[(m, np_, j)] = it
                return inter

            def down(s, inter, wwait=None):
                C_cap, w = caps[s]
                NP_R = C_cap // (2 * w)
                for M in range(MH):
                    wdt = wdp.tile([P, MI, P], bf, name=f"wdt{s}_{M}", tag="wdt")
                    with tc.tile_wait_until(wwait, enable=wwait is not None):
                        nc.sync.dma_start(
                            wdt[:],
                            wd[s * MH + M].rearrange("p (ko c) -> p ko c", c=P))
                    for np_ in range(NP_R):
                        b0 = slot_base[s] + np_ * 2 * w
                        ot = otp.tile([P, 1024], bf,
                                      name=f"ot{s}_{M}_{np_}", tag="ot")
                        for j in range(2):
                            psy = pyp.tile([P, 512], f32,
                                           name=f"psy{s}_{M}_{np_}{j}",
                                           tag="py")
                            for K in range(MI):
                                nc.tensor.matmul(
                                    psy[:, :w], wdt[:, K, :],
                                    inter[(K, np_, j)][:, :w],
                                    start=(K == 0), stop=(K == MI - 1))
                            nc.vector.tensor_copy(
                                ot[:, j * w:(j + 1) * w], psy[:, :w])
                        nc.gpsimd.dma_start(
                            ye[M * P:(M + 1) * P, b0:b0 + 2 * w],
                            ot[:, :2 * w])

            # ---------------- routed experts ----------------
            # tile_wait_until floors (compile-time scheduler hints, in ms)
            # keep non-urgent loads out of the warmup window so slot 0's
            # x stream owns the DMA bandwidth; each floor is ~100us before
            # the consumer needs the data
            w0_0 = load_w0(0, split=True)
            xgt0, w1_0 = load_x(0, with_w1=True)
            w0_1 = load_w0(1, wait=0.25)
            inter0 = gate_up(0, xgt0, w0_0, w_pre={1: w1_0})
            xgt1 = load_x(1, wait=0.15)
            down(0, inter0, wwait=0.10)
            inter1 = gate_up(1, xgt1, w0_1)
            down(1, inter1, wwait=0.45)

            # ------- shared expert (D=2: half columns x 1024 tokens) -------
            xst = []
            with tc.tile_wait_until(0.45):
                for kq in range(KH // 4):
                    t = xsp.tile([P, 4, TS], bf, name=f"xs{kq}", tag="xs")
                    nc.scalar.dma_start(
                        t[:], xs[kq].rearrange("p (kk c) -> p kk c", c=TS))
                    xst.append(t)
            sint = {}
            for m in range(MSI):
                sgt = wp.tile([P, KH, P], bf, name=f"sgt{m}", tag="wp")
                nc.sync.dma_start(
                    sgt[:], sg[m].rearrange("p (ko c) -> p ko c", c=P))
                sut = wp.tile([P, KH, P], bf, name=f"sut{m}", tag="wp")
                nc.sync.dma_start(
                    sut[:], su[m].rearrange("p (ko c) -> p ko c", c=P))
                for j in range(2):
                    psg = pgp.tile([P, 512], f32, name=f"psgs{m}{j}", tag="pg")
                    for k in range(KH):
                        nc.tensor.matmul(
                            psg[:], sgt[:, k, :],
                            xst[k // 4][:, k % 4, j * 512:(j + 1) * 512],
                            start=(k == 0), stop=(k == KH - 1))
                    st = tmp.tile([P, 512], bf, name=f"sts{m}{j}", tag="tmp")
                    nc.scalar.activation(st[:], psg[:], AF.Silu)
                    psu = pup.tile([P, 512], f32, name=f"psus{m}{j}", tag="pu")
                    for k in range(KH):
                        nc.tensor.matmul(
                            psu[:], sut[:, k, :],
                            xst[k // 4][:, k % 4, j * 512:(j + 1) * 512],
                            start=(k == 0), stop=(k == KH - 1))
                    it = itp.tile([P, 512], bf, name=f"si{m}{j}", tag="it")
                    nc.vector.tensor_mul(it[:], st[:], psu[:])
                    sint[(m, j)] = it
            for M in range(MH):
                sdt = sdp.tile([P, MSI, P], bf, name=f"sdt{M}", tag="sdt")
                with tc.tile_wait_until(0.72):
                    nc.gpsimd.dma_start(
                        sdt[:], sd[M].rearrange("p (ko c) -> p ko c", c=P))
                ot = otp.tile([P, 1024], bf, name=f"ots{M}", tag="ot")
                for j in range(2):
                    psy = pyp.tile([P, 512], f32, name=f"psys{M}{j}", tag="py")
                    for K in range(MSI):
                        nc.tensor.matmul(psy[:], sdt[:, K, :], sint[(K, j)][:],
                                         start=(K == 0), stop=(K == MSI - 1))
                    nc.vector.tensor_copy(ot[:, j * 512:(j + 1) * 512], psy[:])
                    nc.scalar.dma_start(
                        ys[M * P:(M + 1) * P, j * 512:(j + 1) * 512],
                        ot[:, j * 512:(j + 1) * 512])

    nc.compile()
    return nc


def _get_compiled(T, caps):
    key = (T, tuple(caps))
    if key not in _COMPILED:
        _COMPILED[key] = _build(T, caps)
    return _COMPILED[key]


def _cap_for(maxc):
    maxc = max(int(maxc), 64)
    np_r = max(2, math.ceil(maxc / 2048))
    w = min(512, 2 * math.ceil(maxc / (np_r * 2 * 2)))
    C_cap = np_r * 2 * w
    assert C_cap >= maxc
    return C_cap, w


def kernel(hidden_states, gate_weight, e_score_correction_bias,
           gate_proj, up_proj, down_proj,
           shared_gate_w, shared_up_w, shared_down_w):
    from concourse.bass_utils import run_bass_kernel_spmd

    hs = np.asarray(hidden_states, dtype=np.float32)
    B, S, Hh = hs.shape
    assert Hh == H
    hsf = np.ascontiguousarray(hs.reshape(-1, H))
    T = hsf.shape[0]
    TS = T // 4        # shared-expert token slice (D=2 hybrid shard)
    gate_weight = np.asarray(gate_weight, np.float32)
    bias = np.asarray(e_score_correction_bias, np.float32)
    gate_proj = np.asarray(gate_proj, np.float32)
    up_proj = np.asarray(up_proj, np.float32)
    down_proj = np.asarray(down_proj, np.float32)
    shared_gate_w = np.asarray(shared_gate_w, np.float32)
    shared_up_w = np.asarray(shared_up_w, np.float32)
    shared_down_w = np.asarray(shared_down_w, np.float32)

    # ---- routing on host ----
    topk_idx, topk_w = _gate_host(hsf, gate_weight, bias)
    comb = np.zeros((T, E), np.float32)
    np.add.at(comb, (np.arange(T)[:, None], topk_idx), topk_w)
    sel = np.zeros((T, E), bool)
    sel[np.arange(T)[:, None], topk_idx] = True
    idx_e = [np.nonzero(sel[:, e])[0] for e in range(E)]
    counts = np.array([len(ix) for ix in idx_e])

    # assign experts to (core, slot): slot 0 gets the 8 largest, slot 1 the
    # 8 smallest, so each slot's capacity (uniform across cores under SPMD)
    # hugs its own max count
    order = np.argsort(-counts, kind="stable")
    assign = np.zeros((N_CORES, EXP_PER_CORE), np.int64)
    for c in range(N_CORES):
        assign[c, 0] = order[c]
        assign[c, 1] = order[2 * N_CORES - 1 - c]
    caps = [
        _cap_for(counts[assign[:, 0]].max()),
        _cap_for(counts[assign[:, 1]].max()),
    ]
    slot_base = [0, caps[0][0]]
    C_tot = caps[0][0] + caps[1][0]

    # ---- host-side dispatch (shard + transpose + bf16 cast) ----
    xsT = np.ascontiguousarray(hsf.T).astype(BF16)          # [H, T]

    MI, MH, MSI, KH = I // P, H // P, SI // (2 * P), H // P
    SIH = SI // 2      # 1408 shared-intermediate columns per group

    def tile_gu(wmat, nm):  # [I', H] -> [nm, P, KH*P] : (m, p_h, ko_h*P + c_i)
        return np.ascontiguousarray(
            wmat.reshape(nm, P, KH, P).transpose(0, 3, 2, 1)
        ).reshape(nm, P, KH * P).astype(BF16)

    def tile_dn(wmat, nk):  # [H, I'] -> [MH, P, nk*P] : (M, p_i, Ko_i*P + c_h)
        return np.ascontiguousarray(
            wmat.reshape(MH, P, nk, P).transpose(0, 3, 2, 1)
        ).reshape(MH, P, nk * P).astype(BF16)

    # shared weights: two column groups (cores 0-3 and 4-7); each core also
    # takes a 1024-token slice, so the shared output is a 2-way partial sum
    sg_g = [tile_gu(shared_gate_w[g * SIH:(g + 1) * SIH], MSI) for g in (0, 1)]
    su_g = [tile_gu(shared_up_w[g * SIH:(g + 1) * SIH], MSI) for g in (0, 1)]
    sd_g = [tile_dn(shared_down_w[:, g * SIH:(g + 1) * SIH], MSI)
            for g in (0, 1)]

    def pack_panels(xmat, NP, cols):
        # [H, NP*cols] -> [NP*4, P, 4*cols], tile np*4+kq holds h rows
        # (kq*4+kk)*128+p and cols [np*cols + c]
        a = xmat.reshape(4, 4, P, NP, cols)          # [kq, kk, p, np, c]
        return np.ascontiguousarray(
            a.transpose(3, 0, 2, 1, 4).reshape(NP * 4, P, 4 * cols)
        )

    in_maps = []
    for c in range(N_CORES):
        e0, e1 = assign[c]
        xg_pk = []
        for sslot, e in enumerate((e0, e1)):
            C_cap, w = caps[sslot]
            NP_R = C_cap // (2 * w)
            xg_c = np.zeros((H, C_cap), BF16)
            xg_c[:, :counts[e]] = xsT[:, idx_e[e]]
            xg_pk.append(pack_panels(xg_c, NP_R, 2 * w))
        wg_c = np.concatenate([tile_gu(gate_proj[e], MI) for e in (e0, e1)])
        wu_c = np.concatenate([tile_gu(up_proj[e], MI) for e in (e0, e1)])
        wd_c = np.concatenate([tile_dn(down_proj[e], MI) for e in (e0, e1)])
        g, ts = c // 4, c % 4
        in_maps.append({
            "xs": pack_panels(
                np.ascontiguousarray(xsT[:, ts * TS:(ts + 1) * TS]), 1, TS),
            "xg0": xg_pk[0], "xg1": xg_pk[1],
            "wg": wg_c, "wu": wu_c, "wd": wd_c,
            "sg": sg_g[g], "su": su_g[g], "sd": sd_g[g],
        })

    nc = _get_compiled(T, caps)
    results = run_bass_kernel_spmd(nc, in_maps, core_ids=list(range(N_CORES)))

    _LAST.clear()
    _LAST.update(nc=nc, in_maps=in_maps, results=results, caps=caps)

    # ---- host-side combine ----
    outT = np.zeros((H, T), np.float32)
    for c in range(N_CORES):
        ts = c % 4
        outT[:, ts * TS:(ts + 1) * TS] += results.results[c]["ys"].astype(np.float32)
    for c in range(N_CORES):
        ye = results.results[c]["ye"].astype(np.float32)
        for sslot in range(EXP_PER_CORE):
            e = assign[c, sslot]
            cnt = counts[e]
            if cnt == 0:
                continue
            b0 = slot_base[sslot]
            we = comb[idx_e[e], e]
            outT[:, idx_e[e]] += ye[:, b0:b0 + cnt] * we[None, :]

    return np.ascontiguousarray(outT.T).reshape(B, S, H).astype(np.float32)
